# revision 1
# baseline (speedup 1.0000x reference)
"""CheckNodeTrellis kernel for Trainium2 (Bass/Tile), 8-core data-parallel.

Math: res[b1,b2,u1,s0,s2] = logsumexp_{u2,s1}( e1[b1,b2,(u1+u2)%2,s0,s1]
                                              + e2[b1,b2,u2,s1,s2] )
which factorizes into exp-space matmuls:
    res[u1] = log( sum_{u2} exp(e1[(u1+u2)%2]) @ exp(e2[u2]) )

Per (b1,b2) pair, both u1 outputs are packed into ONE 128x128x64 matmul:
    out[(u1,s0), s2] = sum_{(u2,s1)} W[(u2,s1),(u1,s0)] * Y[(u2,s1), s2]
with W[(u2,s1),(u1,s0)] = exp(e1[(u1+u2)%2, s0, s1]) and Y = exp(e2) in
natural layout. W is built by one full 128x128 PE transpose of
X2 = [X | Xswap] (Xswap = X with partition halves swapped, materialized
once per b1-batch by two SBUF->SBUF DMAs), exp applied on the
PSUM->SBUF copy.

Sharding: batch axis B1 (16) split across 8 cores, 2 B1-slices per core.
No cross-core communication.
"""

from contextlib import ExitStack

import numpy as np

import concourse.bacc as bacc
import concourse.bass as bass
import concourse.tile as tile
from concourse import mybir
from concourse.masks import make_identity

# Pin Exp and Ln to the one ACT table set that contains both
# (natural_log_exp_and_others). Without this, Bacc's greedy table-load
# insertion alternates between exp_and_others and natural_log_* on every
# exp<->ln switch: 58 LoadActFuncSet x 1.28us = 74us of a 109us kernel.
_orig_get_tables = bacc.get_activation_tables


def _pinned_tables(arch):
    exp_ln = {mybir.ActivationFunctionType.Exp, mybir.ActivationFunctionType.Ln}
    out = {}
    for name, fns in _orig_get_tables(arch).items():
        if name != "natural_log_exp_and_others":
            fns = set(fns) - exp_ln
        out[name] = fns
    return out


bacc.get_activation_tables = _pinned_tables

F32 = mybir.dt.float32
BF16 = mybir.dt.bfloat16
N_CORES = 8
B1, B2, NU, S0, K = 16, 16, 2, 64, 64  # full-problem shape
B1S = B1 // N_CORES  # B1 per core


def make_pools(ctx: ExitStack, tc: "tile.TileContext"):
    nc = tc.nc
    singles = ctx.enter_context(tc.tile_pool(name="singles", bufs=1))
    bigs = ctx.enter_context(tc.tile_pool(name="bigs", bufs=2))
    pairs = ctx.enter_context(tc.tile_pool(name="pairs", bufs=4))
    psums = ctx.enter_context(tc.tile_pool(name="psums", bufs=2, space="PSUM"))
    ident = singles.tile([128, 128], F32)
    make_identity(nc, ident)
    return {"bigs": bigs, "pairs": pairs, "psums": psums, "ident": ident}


def _trellis_body(ctx: ExitStack, tc: "tile.TileContext", out, e1, e2, pools=None):
    nc = tc.nc
    Exp = mybir.ActivationFunctionType.Exp
    Ln = mybir.ActivationFunctionType.Ln

    # Partition-major views: partition = (u, s0) or (u, s1) -> 128 rows,
    # contiguous in DRAM (u stride = 64*64 = 64 * s0 stride).
    e1v = e1.rearrange("b1 b2 u s0 s1 -> b1 (u s0) b2 s1")
    e2v = e2.rearrange("b1 b2 u s1 s2 -> b1 (u s1) b2 s2")
    outv = out.rearrange("b1 b2 u s0 s2 -> b1 (u s0) b2 s2")

    if pools is None:
        pools = make_pools(ctx, tc)
    bigs, pairs, psums = pools["bigs"], pools["pairs"], pools["psums"]
    ident = pools["ident"]

    for b1 in range(B1S):
        # E1[(u,s0), b2, s1]: natural partition-major load, contiguous DRAM.
        E1 = bigs.tile([128, B2, K], F32, tag="E1")
        nc.sync.dma_start(out=E1, in_=e1v[b1])
        # e2 split into u2 halves so both rhs operands sit at partition base 0.
        E2a = bigs.tile([64, B2, K], F32, tag="E2a")
        nc.sync.dma_start(out=E2a, in_=e2v[b1][0:64])
        E2b = bigs.tile([64, B2, K], F32, tag="E2b")
        nc.sync.dma_start(out=E2b, in_=e2v[b1][64:128])
        Y0 = bigs.tile([64, B2, K], BF16, tag="Y0")
        nc.scalar.activation(out=Y0, in_=E2a, func=Exp)
        Y1 = bigs.tile([64, B2, K], BF16, tag="Y1")
        nc.scalar.activation(out=Y1, in_=E2b, func=Exp)
        OUT = bigs.tile([128, B2, K], F32, tag="OUT")

        # Groups of G pairs: batched exp/log amortize the ~220c ACT
        # per-instruction overhead.
        G = 8
        for g in range(B2 // G):
            # TTg[s1, j, (u1,s0)] = e1[pair][u1, s0, s1]  (pure transpose)
            TTg = psums.tile([64, G, 128], F32, tag="TTg")
            for j in range(G):
                nc.tensor.transpose(TTg[:, j, :], E1[:, g * G + j, :], ident)
            Wg = pairs.tile([64, G, 128], BF16, tag="Wg")
            nc.scalar.activation(out=Wg, in_=TTg, func=Exp)
            # Accumulate over u2 with base-0 K=64 matmuls:
            #  u2=0: R[(u1,s0)] += expP_{u1}^T.T  @ expQ0   (full M=128)
            #  u2=1: R[(0,s0)]  += expP_1^T.T @ expQ1  (W free-slice 64:128)
            #        R[(1,s0)]  += expP_0^T.T @ expQ1  (W free-slice 0:64)
            Rg = psums.tile([128, G, K], F32, tag="Rg")
            for j in range(G):
                b2 = g * G + j
                nc.tensor.matmul(Rg[:, j, :], Wg[:, j, :], Y0[:, b2, :],
                                 start=True, stop=False)
                nc.tensor.matmul(Rg[0:64, j, :], Wg[:, j, K:], Y1[:, b2, :],
                                 start=False, stop=False)
                nc.tensor.matmul(Rg[64:128, j, :], Wg[:, j, 0:K], Y1[:, b2, :],
                                 start=False, stop=True)
            nc.scalar.activation(out=OUT[:, g * G:(g + 1) * G, :], in_=Rg,
                                 func=Ln)

        nc.sync.dma_start(out=outv[b1], in_=OUT)


def build_nc(num_devices: int = N_CORES) -> bass.Bass:
    nc = bacc.Bacc("TRN2", target_bir_lowering=False, debug=False,
                   num_devices=num_devices)
    e1 = nc.dram_tensor("e1", [B1S, B2, NU, S0, K], F32, kind="ExternalInput").ap()
    e2 = nc.dram_tensor("e2", [B1S, B2, NU, K, K], F32, kind="ExternalInput").ap()
    out = nc.dram_tensor("out", [B1S, B2, NU, S0, K], F32, kind="ExternalOutput").ap()
    with tile.TileContext(nc) as tc:
        with ExitStack() as ctx:
            _trellis_body(ctx, tc, out, e1, e2)
    nc.compile()
    return nc


_NC_CACHE = None


def kernel(e1: np.ndarray, e2: np.ndarray) -> np.ndarray:
    from concourse import bass_utils

    global _NC_CACHE
    e1 = np.ascontiguousarray(np.asarray(e1, dtype=np.float32))
    e2 = np.ascontiguousarray(np.asarray(e2, dtype=np.float32))
    assert e1.shape == (B1, B2, NU, S0, K), e1.shape
    assert e2.shape == (B1, B2, NU, K, K), e2.shape

    if _NC_CACHE is None:
        _NC_CACHE = build_nc()
    nc = _NC_CACHE

    in_maps = []
    for c in range(N_CORES):
        sl = slice(c * B1S, (c + 1) * B1S)
        in_maps.append({
            "e1": np.ascontiguousarray(e1[sl]),
            "e2": np.ascontiguousarray(e2[sl]),
        })
    res = bass_utils.run_bass_kernel_spmd(nc, in_maps, core_ids=list(range(N_CORES)))
    return np.concatenate([r["out"] for r in res.results], axis=0)



# revision 2
# speedup vs baseline: 24.1168x; 24.1168x over previous
"""CheckNodeTrellis kernel for Trainium2 (Bass/Tile), 8-core data-parallel.

Math: res[b1,b2,u1,s0,s2] = logsumexp_{u2,s1}( e1[b1,b2,(u1+u2)%2,s0,s1]
                                              + e2[b1,b2,u2,s1,s2] )
which factorizes into exp-space matmuls:
    res[u1] = log( sum_{u2} exp(e1[(u1+u2)%2]) @ exp(e2[u2]) )

Per (b1,b2) pair, both u1 outputs are packed into ONE 128x128x64 matmul:
    out[(u1,s0), s2] = sum_{(u2,s1)} W[(u2,s1),(u1,s0)] * Y[(u2,s1), s2]
with W[(u2,s1),(u1,s0)] = exp(e1[(u1+u2)%2, s0, s1]) and Y = exp(e2) in
natural layout. W is built by one full 128x128 PE transpose of
X2 = [X | Xswap] (Xswap = X with partition halves swapped, materialized
once per b1-batch by two SBUF->SBUF DMAs), exp applied on the
PSUM->SBUF copy.

Sharding: batch axis B1 (16) split across 8 cores, 2 B1-slices per core.
No cross-core communication.
"""

from contextlib import ExitStack

import numpy as np

import concourse.bacc as bacc
import concourse.bass as bass
import concourse.tile as tile
from concourse import mybir
from concourse.masks import make_identity

# Pin Exp and Ln to the one ACT table set that contains both
# (natural_log_exp_and_others). Without this, Bacc's greedy table-load
# insertion alternates between exp_and_others and natural_log_* on every
# exp<->ln switch: 58 LoadActFuncSet x 1.28us = 74us of a 109us kernel.
_orig_get_tables = bacc.get_activation_tables


def _pinned_tables(arch):
    exp_ln = {mybir.ActivationFunctionType.Exp, mybir.ActivationFunctionType.Ln}
    out = {}
    for name, fns in _orig_get_tables(arch).items():
        if name != "natural_log_exp_and_others":
            fns = set(fns) - exp_ln
        out[name] = fns
    return out


bacc.get_activation_tables = _pinned_tables

F32 = mybir.dt.float32
BF16 = mybir.dt.bfloat16
N_CORES = 8
B1, B2, NU, S0, K = 16, 16, 2, 64, 64  # full-problem shape
B1S = B1 // N_CORES  # B1 per core


def make_pools(ctx: ExitStack, tc: "tile.TileContext"):
    nc = tc.nc
    singles = ctx.enter_context(tc.tile_pool(name="singles", bufs=1))
    bigs = ctx.enter_context(tc.tile_pool(name="bigs", bufs=2))
    pairs = ctx.enter_context(tc.tile_pool(name="pairs", bufs=4))
    psums = ctx.enter_context(tc.tile_pool(name="psums", bufs=2, space="PSUM"))
    ident = singles.tile([128, 128], F32)
    make_identity(nc, ident)
    return {"bigs": bigs, "pairs": pairs, "psums": psums, "ident": ident}


def _trellis_body(ctx: ExitStack, tc: "tile.TileContext", out, e1, e2, pools=None):
    nc = tc.nc
    Exp = mybir.ActivationFunctionType.Exp
    Ln = mybir.ActivationFunctionType.Ln

    # Partition-major views: partition = (u, s0) or (u, s1) -> 128 rows,
    # contiguous in DRAM (u stride = 64*64 = 64 * s0 stride).
    e1v = e1.rearrange("b1 b2 u s0 s1 -> b1 (u s0) b2 s1")
    e2v = e2.rearrange("b1 b2 u s1 s2 -> b1 (u s1) b2 s2")
    outv = out.rearrange("b1 b2 u s0 s2 -> b1 (u s0) b2 s2")

    if pools is None:
        pools = make_pools(ctx, tc)
    bigs, pairs, psums = pools["bigs"], pools["pairs"], pools["psums"]
    ident = pools["ident"]

    for b1 in range(B1S):
        # E1[(u,s0), b2, s1]: natural partition-major load, contiguous DRAM.
        E1 = bigs.tile([128, B2, K], F32, tag="E1")
        nc.sync.dma_start(out=E1, in_=e1v[b1])
        # e2 split into u2 halves so both rhs operands sit at partition base 0.
        E2a = bigs.tile([64, B2, K], F32, tag="E2a")
        nc.sync.dma_start(out=E2a, in_=e2v[b1][0:64])
        E2b = bigs.tile([64, B2, K], F32, tag="E2b")
        nc.sync.dma_start(out=E2b, in_=e2v[b1][64:128])
        Y0 = bigs.tile([64, B2, K], BF16, tag="Y0")
        nc.scalar.activation(out=Y0, in_=E2a, func=Exp)
        Y1 = bigs.tile([64, B2, K], BF16, tag="Y1")
        nc.scalar.activation(out=Y1, in_=E2b, func=Exp)
        OUT = bigs.tile([128, B2, K], F32, tag="OUT")

        # Groups of G pairs: batched exp/log amortize the ~220c ACT
        # per-instruction overhead.
        G = 8
        for g in range(B2 // G):
            # TTg[s1, j, (u1,s0)] = e1[pair][u1, s0, s1]  (pure transpose)
            TTg = psums.tile([64, G, 128], F32, tag="TTg")
            for j in range(G):
                nc.tensor.transpose(TTg[:, j, :], E1[:, g * G + j, :], ident)
            Wg = pairs.tile([64, G, 128], BF16, tag="Wg")
            nc.scalar.activation(out=Wg, in_=TTg, func=Exp)
            # Accumulate over u2 with base-0 K=64 matmuls:
            #  u2=0: R[(u1,s0)] += expP_{u1}^T.T  @ expQ0   (full M=128)
            #  u2=1: R[(0,s0)]  += expP_1^T.T @ expQ1  (W free-slice 64:128)
            #        R[(1,s0)]  += expP_0^T.T @ expQ1  (W free-slice 0:64)
            Rg = psums.tile([128, G, K], F32, tag="Rg")
            for j in range(G):
                b2 = g * G + j
                nc.tensor.matmul(Rg[:, j, :], Wg[:, j, :], Y0[:, b2, :],
                                 start=True, stop=False, skip_group_check=True)
                nc.tensor.matmul(Rg[0:64, j, :], Wg[:, j, K:], Y1[:, b2, :],
                                 start=False, stop=False, skip_group_check=True)
                nc.tensor.matmul(Rg[64:128, j, :], Wg[:, j, 0:K], Y1[:, b2, :],
                                 start=False, stop=True, skip_group_check=True)
            nc.scalar.activation(out=OUT[:, g * G:(g + 1) * G, :], in_=Rg,
                                 func=Ln)

        nc.sync.dma_start(out=outv[b1], in_=OUT)


def build_nc(num_devices: int = N_CORES) -> bass.Bass:
    nc = bacc.Bacc("TRN2", target_bir_lowering=False, debug=False,
                   num_devices=num_devices)
    e1 = nc.dram_tensor("e1", [B1S, B2, NU, S0, K], F32, kind="ExternalInput").ap()
    e2 = nc.dram_tensor("e2", [B1S, B2, NU, K, K], F32, kind="ExternalInput").ap()
    out = nc.dram_tensor("out", [B1S, B2, NU, S0, K], F32, kind="ExternalOutput").ap()
    with tile.TileContext(nc) as tc:
        with ExitStack() as ctx:
            _trellis_body(ctx, tc, out, e1, e2)
    nc.compile()
    return nc


_NC_CACHE = None


def kernel(e1: np.ndarray, e2: np.ndarray) -> np.ndarray:
    from concourse import bass_utils

    global _NC_CACHE
    e1 = np.ascontiguousarray(np.asarray(e1, dtype=np.float32))
    e2 = np.ascontiguousarray(np.asarray(e2, dtype=np.float32))
    assert e1.shape == (B1, B2, NU, S0, K), e1.shape
    assert e2.shape == (B1, B2, NU, K, K), e2.shape

    if _NC_CACHE is None:
        _NC_CACHE = build_nc()
    nc = _NC_CACHE

    in_maps = []
    for c in range(N_CORES):
        sl = slice(c * B1S, (c + 1) * B1S)
        in_maps.append({
            "e1": np.ascontiguousarray(e1[sl]),
            "e2": np.ascontiguousarray(e2[sl]),
        })
    res = bass_utils.run_bass_kernel_spmd(nc, in_maps, core_ids=list(range(N_CORES)))
    return np.concatenate([r["out"] for r in res.results], axis=0)

